# revision 4
# baseline (speedup 1.0000x reference)
"""Trainium2 Bass kernel for nn_CAM_Multimodal_Module (retrieval_knn).

Per batch b:
    energy[i, j] = <rgb[b, i, :], depth[b, j, :]>   (contraction over H*W)
    cl[i] = argmax_j energy[i, j]
    out[b, i, :] = rgb[b, i, :] + depth[b, cl[i], :]

Sharding: pure data parallel, 2 batches per core across 8 cores.

Energy path ("fp16_fp8dr", default): split q = qh + ql and k = kh + kl into
fp16 halves, then compute (at a global 2^11 scale, argmax-invariant)
    2^11 E ~= qh.(2^11 kh)  [fp16 matmul, 1 cyc/row]
            + [qh | 2^11 ql] .DR. [2^11 kl | kh]   [fp8 e4m3 DoubleRow, 0.5 cyc/row]
all accumulating into ONE PSUM bank per channel tile. The DoubleRow matmul
computes both cross terms (qh.kl + ql.kh, each carrying exactly one 2^11
factor) in a single 256-deep pass, halving the PE cost of the correction vs
the old fp16x3 scheme (verified offline on these inputs: 0 argmax flips,
top-2 margin 1.19e-3 vs fp64-truth gap 1.27e-3). Dropped ql.kl term is
O(1e-6). The scale assignment keeps every fp8 operand in e4m3's sweet range
(typ 0.3-1) with no separate-combine pass.

The exact add uses the original fp32 data: argmax indices drive gpsimd
indirect DMAs that gather exact fp32 depth rows from DRAM with a CCE add,
accumulating in-flight onto the fp32 rgb tiles (transfers are chunked to
4608 B -- larger indirect-DMA-with-add transfers corrupt on HW).

Set ENERGY_DT = "fp16x3" for the previous 3-matmul fp16 scheme.
"""

import numpy as np
from contextlib import ExitStack

import concourse.bass as bass
import concourse.tile as tile
from concourse import bacc, mybir
from concourse.bass_utils import run_bass_kernel_spmd
from concourse.masks import make_identity
from concourse._compat import with_exitstack

B, C, H, W = 16, 512, 48, 48
HW = H * W              # 2304
NCORES = 8
NB = B // NCORES        # 2 batches per core
P = 128
NT = C // P             # 4 channel tiles
NCH = HW // P           # 18 contraction chunks
F32 = mybir.dt.float32
F16 = mybir.dt.float16
F8 = mybir.dt.float8e4
SCALE = 2048.0          # 2^11

ENERGY_DT = "fp16_fp8dr"    # "fp16_fp8dr" | "fp16x3"

_NC_CACHE = {}


def _argmax_gather_store(nc, tc, argp, b, t, energy_t, rgb_t_t, dep_d, out_d):
    mx8 = argp.tile([P, 8], F32, tag="mx8", name=f"mx8_b{b}t{t}")
    nc.vector.max(mx8[:], energy_t[:])
    idx8 = argp.tile([P, 8], mybir.dt.uint32, tag="idx8", name=f"idx8_b{b}t{t}")
    nc.vector.max_index(idx8[:], mx8[:], energy_t[:])
    # gather exact fp32 depth rows from DRAM, accumulating onto the rgb tile
    # in-flight (CCE add). Transfers > 4608 B corrupt on HW, so chunk by 1152.
    half = HW // 2
    for c0 in (0, half):
        nc.gpsimd.indirect_dma_start(
            out=rgb_t_t[:, c0 : c0 + half],
            out_offset=None,
            in_=dep_d[:],
            in_offset=bass.IndirectOffsetOnAxis(ap=idx8[:, 0:1], axis=0),
            element_offset=b * C * HW + c0,
            compute_op=mybir.AluOpType.add,
        )
    store_eng = nc.sync if t % 2 == 0 else nc.scalar
    store_eng.dma_start(out_d[b * C + t * P : b * C + (t + 1) * P, :], rgb_t_t[:])


@with_exitstack
def _body_fp16_fp8dr(ctx, tc, out_d, rgb_d, dep_d):
    nc = tc.nc
    consts = ctx.enter_context(tc.tile_pool(name="consts", bufs=1))
    rgbp = ctx.enter_context(tc.tile_pool(name="rgbp", bufs=2))
    depp = ctx.enter_context(tc.tile_pool(name="depp", bufs=2))
    splitp = ctx.enter_context(tc.tile_pool(name="splitp", bufs=1))
    tpose = ctx.enter_context(tc.tile_pool(name="tpose", bufs=4))
    psum_t = ctx.enter_context(tc.tile_pool(name="psum_t", bufs=2, space="PSUM"))
    psum_e = ctx.enter_context(tc.tile_pool(name="psum_e", bufs=1, space="PSUM"))
    argp = ctx.enter_context(tc.tile_pool(name="argp", bufs=2))

    ident = consts.tile([P, P], F16, tag="ident")
    make_identity(nc, ident[:])

    # load/split pieces: a small head piece so the chunk loop starts early,
    # then the remainder. Subtile deps let chunk ch wait only on its piece.
    PIECES = [(0, 512), (512, HW - 512)]

    def emit_loads(b):
        rgb_t = []
        for t in range(NT):
            r = rgbp.tile([P, HW], F32, tag=f"rgb{t}", name=f"rgb_b{b}t{t}")
            rgb_t.append(r)
        for c0, w in PIECES:
            for t in range(NT):
                row = b * C + t * P
                nc.sync.dma_start(
                    rgb_t[t][:, c0 : c0 + w], rgb_d[row : row + P, c0 : c0 + w]
                )
        return rgb_t

    def emit_splits(b, rgb_t):
        qh_t, ql_t, kh_t, kl_t = [], [], [], []
        dls = []
        for t in range(NT):
            d = depp.tile([P, HW], F32, tag=f"dep{t % 2}", name=f"dep_b{b}t{t}")
            dls.append(d)
            qh_t.append(splitp.tile([P, HW], F16, tag=f"qh{t}", name=f"qh_b{b}t{t}"))
            ql_t.append(splitp.tile([P, HW], F16, tag=f"ql{t}", name=f"ql_b{b}t{t}"))
            kh_t.append(splitp.tile([P, HW], F16, tag=f"kh{t}", name=f"kh_b{b}t{t}"))
            kl_t.append(splitp.tile([P, HW], F16, tag=f"kl{t}", name=f"kl_b{b}t{t}"))
        for c0, w in PIECES:
            for t in range(NT):
                row = b * C + t * P
                nc.scalar.dma_start(
                    dls[t][:, c0 : c0 + w], dep_d[row : row + P, c0 : c0 + w]
                )
        for c0, w in PIECES:
            ps = slice(c0, c0 + w)
            for t in range(NT):
                # fp16 splits: xh = fp16(x) ; xl = fp16(x - xh)
                # engine balance: qh+kh on ACT, ql+kl on DVE
                nc.scalar.copy(qh_t[t][:, ps], rgb_t[t][:, ps])
                nc.vector.tensor_sub(ql_t[t][:, ps], rgb_t[t][:, ps], qh_t[t][:, ps])
                nc.scalar.copy(kh_t[t][:, ps], dls[t][:, ps])
                nc.vector.tensor_sub(kl_t[t][:, ps], dls[t][:, ps], kh_t[t][:, ps])
        return qh_t, ql_t, kh_t, kl_t

    def emit_chunks(b, halves):
        qh_t, ql_t, kh_t, kl_t = halves
        energy = [
            psum_e.tile([P, C], F32, tag=f"energy{t}", name=f"energy_b{b}t{t}")
            for t in range(NT)
        ]
        ops = [None] * NCH

        def emit_transposes(ch):
            cs = slice(ch * P, (ch + 1) * P)
            # staging (plain transposes -- the PE transpose is pure data
            # movement, identity values are NOT applied):
            #   ps_q = [qhT | qlT], ps_k = [klT | khT]
            ps_q = psum_t.tile([P, 2 * C], F16, tag="ps_q", name=f"ps_q_b{b}c{ch}")
            ps_k = psum_t.tile([P, 2 * C], F16, tag="ps_k", name=f"ps_k_b{b}c{ch}")
            for t in range(NT):
                nc.tensor.transpose(ps_q[:, t * P : (t + 1) * P], qh_t[t][:, cs], ident[:])
                nc.tensor.transpose(ps_q[:, C + t * P : C + (t + 1) * P], ql_t[t][:, cs], ident[:])
                nc.tensor.transpose(ps_k[:, t * P : (t + 1) * P], kl_t[t][:, cs], ident[:])
                nc.tensor.transpose(ps_k[:, C + t * P : C + (t + 1) * P], kh_t[t][:, cs], ident[:])
            # fp16 operands for the main matmul; 2^11 rides on kh16 so that
            # main accumulates at the same 2^11 scale as the fp8 crosses.
            qh16 = tpose.tile([P, C], F16, tag="qh16", bufs=5, name=f"qh16_b{b}c{ch}")
            kh16 = tpose.tile([P, C], F16, tag="kh16", bufs=5, name=f"kh16_b{b}c{ch}")
            nc.vector.tensor_copy(qh16[:], ps_q[:, 0:C])
            nc.scalar.mul(kh16[:], ps_k[:, C : 2 * C], SCALE)
            # fp8 packs for the DoubleRow cross matmul. Low halves are cast
            # from PSUM with the 2^11 scale applied in the cast (DVE/ACT);
            # high halves are cast on gpsimd from the fp16 SBUF copies
            # (SBUF->SBUF -- gpsimd has no PSUM port), undoing kh16's 2^11.
            q8 = tpose.tile([P, 2 * C], F8, tag="q8", bufs=4, name=f"q8_b{b}c{ch}")
            k8 = tpose.tile([P, 2 * C], F8, tag="k8", bufs=4, name=f"k8_b{b}c{ch}")
            nc.vector.tensor_scalar_mul(q8[:, C : 2 * C], ps_q[:, C : 2 * C], SCALE)
            nc.scalar.mul(k8[:, 0:C], ps_k[:, 0:C], SCALE)
            nc.gpsimd.tensor_copy(q8[:, 0:C], qh16[:])
            nc.gpsimd.tensor_scalar_mul(k8[:, C : 2 * C], kh16[:], 1.0 / SCALE)
            ops[ch] = (qh16, kh16, q8, k8)

        def emit_matmuls(ch, tiles=range(NT)):
            qh16, kh16, q8, k8 = ops[ch]
            q83 = q8[:].rearrange("p (two m) -> p two m", two=2)
            k83 = k8[:].rearrange("p (two n) -> p two n", two=2)
            for t in tiles:
                nc.tensor.matmul(
                    energy[t][:],
                    lhsT=qh16[:, t * P : (t + 1) * P],
                    rhs=kh16[:],
                    start=(ch == 0),
                    stop=False,
                )
                nc.tensor.matmul(
                    energy[t][:],
                    lhsT=q83[:, :, t * P : (t + 1) * P],
                    rhs=k83,
                    start=False,
                    stop=(ch == NCH - 1),
                    perf_mode=mybir.MatmulPerfMode.DoubleRow,
                )

        TMAJ = 6  # tile-major over the last TMAJ chunks
        emit_transposes(0)
        emit_transposes(1)
        emit_transposes(2)
        for ch in range(3, NCH):
            emit_transposes(ch)
            if ch - 3 < NCH - TMAJ:
                emit_matmuls(ch - 3)
        if NCH - 3 < NCH - TMAJ:
            emit_matmuls(NCH - 3)
        # tile-major for the last chunks: tile t's accumulation finishes early
        # so its argmax/gather/store overlaps the remaining matmuls.
        for t in range(NT):
            for ch in range(NCH - TMAJ, NCH):
                emit_matmuls(ch, tiles=[t])
        return energy

    def emit_tail(b, energy, rgb_t):
        for t in range(NT):
            _argmax_gather_store(
                nc, tc, argp, b, t, energy[t], rgb_t[t], dep_d, out_d
            )

    # phase-ordered emission: prefetch b1 loads early; emit b1 splits before
    # b0's tail so the DVE un-blocks the PE first; b0's tail overlaps b1's
    # chunk phase.
    rgb0 = emit_loads(0)
    halves0 = emit_splits(0, rgb0)
    rgb1 = emit_loads(1)
    energy0 = emit_chunks(0, halves0)
    halves1 = emit_splits(1, rgb1)
    energy1 = emit_chunks(1, halves1)
    emit_tail(0, energy0, rgb0)
    emit_tail(1, energy1, rgb1)


@with_exitstack
def _body_fp16x3(ctx, tc, out_d, rgb_d, dep_d):
    nc = tc.nc
    consts = ctx.enter_context(tc.tile_pool(name="consts", bufs=1))
    rgbp = ctx.enter_context(tc.tile_pool(name="rgbp", bufs=2))
    depp = ctx.enter_context(tc.tile_pool(name="depp", bufs=2))
    splitp = ctx.enter_context(tc.tile_pool(name="splitp", bufs=1))
    tpose = ctx.enter_context(tc.tile_pool(name="tpose", bufs=5))
    psum_t = ctx.enter_context(tc.tile_pool(name="psum_t", bufs=2, space="PSUM"))
    psum_e = ctx.enter_context(tc.tile_pool(name="psum_e", bufs=1, space="PSUM"))
    argp = ctx.enter_context(tc.tile_pool(name="argp", bufs=2))

    ident = consts.tile([P, P], F16, tag="ident")
    make_identity(nc, ident[:])

    PIECES = [(0, 512), (512, HW - 512)]

    def emit_loads(b):
        rgb_t = []
        for t in range(NT):
            r = rgbp.tile([P, HW], F32, tag=f"rgb{t}", name=f"rgb_b{b}t{t}")
            rgb_t.append(r)
        for c0, w in PIECES:
            for t in range(NT):
                row = b * C + t * P
                nc.sync.dma_start(
                    rgb_t[t][:, c0 : c0 + w], rgb_d[row : row + P, c0 : c0 + w]
                )
        return rgb_t

    def emit_splits(b, rgb_t):
        qh_t, ql_t, kh_t, kl_t = [], [], [], []
        dls = []
        for t in range(NT):
            d = depp.tile([P, HW], F32, tag=f"dep{t % 2}", name=f"dep_b{b}t{t}")
            dls.append(d)
            qh_t.append(splitp.tile([P, HW], F16, tag=f"qh{t}", name=f"qh_b{b}t{t}"))
            ql_t.append(splitp.tile([P, HW], F16, tag=f"ql{t}", name=f"ql_b{b}t{t}"))
            kh_t.append(splitp.tile([P, HW], F16, tag=f"kh{t}", name=f"kh_b{b}t{t}"))
            kl_t.append(splitp.tile([P, HW], F16, tag=f"kl{t}", name=f"kl_b{b}t{t}"))
        for c0, w in PIECES:
            for t in range(NT):
                row = b * C + t * P
                nc.scalar.dma_start(
                    dls[t][:, c0 : c0 + w], dep_d[row : row + P, c0 : c0 + w]
                )
        for c0, w in PIECES:
            ps = slice(c0, c0 + w)
            for t in range(NT):
                nc.scalar.copy(qh_t[t][:, ps], rgb_t[t][:, ps])
                nc.vector.tensor_sub(ql_t[t][:, ps], rgb_t[t][:, ps], qh_t[t][:, ps])
                nc.scalar.copy(kh_t[t][:, ps], dls[t][:, ps])
                nc.gpsimd.tensor_sub(kl_t[t][:, ps], dls[t][:, ps], kh_t[t][:, ps])
        return qh_t, ql_t, kh_t, kl_t

    def emit_chunks(b, halves):
        qh_t, ql_t, kh_t, kl_t = halves
        energy = [
            psum_e.tile([P, C], F32, tag=f"energy{t}", name=f"energy_b{b}t{t}")
            for t in range(NT)
        ]
        qkT = [None] * NCH

        def emit_transposes(ch):
            cs = slice(ch * P, (ch + 1) * P)
            ps_q = psum_t.tile([P, 2 * C], F16, tag="ps_q", name=f"ps_q_b{b}c{ch}")
            ps_k = psum_t.tile([P, 2 * C], F16, tag="ps_k", name=f"ps_k_b{b}c{ch}")
            for t in range(NT):
                nc.tensor.transpose(ps_q[:, t * P : (t + 1) * P], qh_t[t][:, cs], ident[:])
                nc.tensor.transpose(ps_q[:, C + t * P : C + (t + 1) * P], ql_t[t][:, cs], ident[:])
                nc.tensor.transpose(ps_k[:, t * P : (t + 1) * P], kh_t[t][:, cs], ident[:])
                nc.tensor.transpose(ps_k[:, C + t * P : C + (t + 1) * P], kl_t[t][:, cs], ident[:])
            qT = tpose.tile([P, 2 * C], F16, tag="qT", bufs=7, name=f"qT_b{b}c{ch}")
            kT = tpose.tile([P, 2 * C], F16, tag="kT", bufs=6, name=f"kT_b{b}c{ch}")
            nc.vector.tensor_copy(qT[:], ps_q[:])
            nc.vector.tensor_copy(kT[:], ps_k[:])
            qkT[ch] = (qT, kT)

        def emit_matmuls(ch, tiles=range(NT)):
            qT, kT = qkT[ch]
            khT = kT[:, 0:C]
            klT = kT[:, C : 2 * C]
            for t in tiles:
                qhT_t = qT[:, t * P : (t + 1) * P]
                qlT_t = qT[:, C + t * P : C + (t + 1) * P]
                nc.tensor.matmul(energy[t][:], lhsT=qhT_t, rhs=khT,
                                 start=(ch == 0), stop=False)
                nc.tensor.matmul(energy[t][:], lhsT=qhT_t, rhs=klT,
                                 start=False, stop=False)
                nc.tensor.matmul(energy[t][:], lhsT=qlT_t, rhs=khT,
                                 start=False, stop=(ch == NCH - 1))

        TMAJ = 6
        emit_transposes(0)
        emit_transposes(1)
        emit_transposes(2)
        for ch in range(3, NCH):
            emit_transposes(ch)
            if ch - 3 < NCH - TMAJ:
                emit_matmuls(ch - 3)
        if NCH - 3 < NCH - TMAJ:
            emit_matmuls(NCH - 3)
        for t in range(NT):
            for ch in range(NCH - TMAJ, NCH):
                emit_matmuls(ch, tiles=[t])
        return energy

    def emit_tail(b, energy, rgb_t):
        for t in range(NT):
            _argmax_gather_store(
                nc, tc, argp, b, t, energy[t], rgb_t[t], dep_d, out_d
            )

    rgb0 = emit_loads(0)
    halves0 = emit_splits(0, rgb0)
    rgb1 = emit_loads(1)
    energy0 = emit_chunks(0, halves0)
    halves1 = emit_splits(1, rgb1)
    energy1 = emit_chunks(1, halves1)
    emit_tail(0, energy0, rgb0)
    emit_tail(1, energy1, rgb1)


def _build():
    nc = bacc.Bacc("TRN2", target_bir_lowering=False, debug=False)
    rgb_d = nc.dram_tensor("rgb", [NB * C, HW], F32, kind="ExternalInput")
    dep_d = nc.dram_tensor("depth", [NB * C, HW], F32, kind="ExternalInput")
    out_d = nc.dram_tensor("out", [NB * C, HW], F32, kind="ExternalOutput")
    body = _body_fp16_fp8dr if ENERGY_DT == "fp16_fp8dr" else _body_fp16x3
    with tile.TileContext(nc) as tc:
        body(tc, out_d.ap(), rgb_d.ap(), dep_d.ap())
    nc.compile()
    return nc


def get_nc():
    if "nc" not in _NC_CACHE:
        _NC_CACHE["nc"] = _build()
    return _NC_CACHE["nc"]


def make_in_maps(rgb, depth):
    rgb = np.ascontiguousarray(np.asarray(rgb, dtype=np.float32)).reshape(B, C, HW)
    depth = np.ascontiguousarray(np.asarray(depth, dtype=np.float32)).reshape(B, C, HW)
    in_maps = []
    for i in range(NCORES):
        sl = slice(i * NB, (i + 1) * NB)
        in_maps.append(
            {
                "rgb": np.ascontiguousarray(rgb[sl]).reshape(NB * C, HW),
                "depth": np.ascontiguousarray(depth[sl]).reshape(NB * C, HW),
            }
        )
    return in_maps


def kernel(rgb, depth):
    nc = get_nc()
    in_maps = make_in_maps(rgb, depth)
    res = run_bass_kernel_spmd(nc, in_maps, core_ids=list(range(NCORES)))
    outs = [res.results[i]["out"].reshape(NB, C, H, W) for i in range(NCORES)]
    return np.concatenate(outs, axis=0)


# revision 11
# speedup vs baseline: 1.0176x; 1.0176x over previous
"""Trainium2 Bass kernel for nn_CAM_Multimodal_Module (retrieval_knn).

Per batch b:
    energy[i, j] = <rgb[b, i, :], depth[b, j, :]>   (contraction over H*W)
    cl[i] = argmax_j energy[i, j]
    out[b, i, :] = rgb[b, i, :] + depth[b, cl[i], :]

Sharding: pure data parallel, 2 batches per core across 8 cores.

Energy path ("fp16_fp8dr", default): split q = qh + ql and k = kh + kl into
fp16 halves, then compute (at a global 2^11 scale, argmax-invariant)
    2^11 E ~= qh.(2^11 kh)  [fp16 matmul, 1 cyc/row]
            + [qh | 2^11 ql] .DR. [2^11 kl | kh]   [fp8 e4m3 DoubleRow, 0.5 cyc/row]
all accumulating into ONE PSUM bank per channel tile. The DoubleRow matmul
computes both cross terms (qh.kl + ql.kh, each carrying exactly one 2^11
factor) in a single 256-deep pass, halving the PE cost of the correction vs
the old fp16x3 scheme (verified offline on these inputs: 0 argmax flips,
top-2 margin 1.19e-3 vs fp64-truth gap 1.27e-3). Dropped ql.kl term is
O(1e-6). The scale assignment keeps every fp8 operand in e4m3's sweet range
(typ 0.3-1) with no separate-combine pass.

The exact add uses the original fp32 data: argmax indices drive gpsimd
indirect DMAs that gather exact fp32 depth rows from DRAM with a CCE add,
accumulating in-flight onto the fp32 rgb tiles (transfers are chunked to
4608 B -- larger indirect-DMA-with-add transfers corrupt on HW).

Set ENERGY_DT = "fp16x3" for the previous 3-matmul fp16 scheme.
"""

import numpy as np
from contextlib import ExitStack

import concourse.bass as bass
import concourse.tile as tile
from concourse import bacc, mybir
from concourse.bass_utils import run_bass_kernel_spmd
from concourse.masks import make_identity
from concourse._compat import with_exitstack

B, C, H, W = 16, 512, 48, 48
HW = H * W              # 2304
NCORES = 8
NB = B // NCORES        # 2 batches per core
P = 128
NT = C // P             # 4 channel tiles
NCH = HW // P           # 18 contraction chunks
F32 = mybir.dt.float32
F16 = mybir.dt.float16
F8 = mybir.dt.float8e4
SCALE = 2048.0          # 2^11

ENERGY_DT = "fp16_fp8dr"    # "fp16_fp8dr" | "fp16x3"

_NC_CACHE = {}


def _argmax_gather_store(nc, tc, argp, b, t, energy_t, rgb_t_t, dep_d, out_d):
    mx8 = argp.tile([P, 8], F32, tag="mx8", name=f"mx8_b{b}t{t}")
    nc.vector.max(mx8[:], energy_t[:])
    idx8 = argp.tile([P, 8], mybir.dt.uint32, tag="idx8", name=f"idx8_b{b}t{t}")
    nc.vector.max_index(idx8[:], mx8[:], energy_t[:])
    # gather exact fp32 depth rows from DRAM, accumulating onto the rgb tile
    # in-flight (CCE add). Transfers > 4608 B corrupt on HW, so chunk by 1152.
    half = HW // 2
    for c0 in (0, half):
        nc.gpsimd.indirect_dma_start(
            out=rgb_t_t[:, c0 : c0 + half],
            out_offset=None,
            in_=dep_d[:],
            in_offset=bass.IndirectOffsetOnAxis(ap=idx8[:, 0:1], axis=0),
            element_offset=b * C * HW + c0,
            compute_op=mybir.AluOpType.add,
        )
    store_eng = nc.sync if t % 2 == 0 else nc.scalar
    store_eng.dma_start(out_d[b * C + t * P : b * C + (t + 1) * P, :], rgb_t_t[:])


@with_exitstack
def _body_fp16_fp8dr(ctx, tc, out_d, rgb_d, dep_d):
    nc = tc.nc
    consts = ctx.enter_context(tc.tile_pool(name="consts", bufs=1))
    rgbp = ctx.enter_context(tc.tile_pool(name="rgbp", bufs=2))
    depp = ctx.enter_context(tc.tile_pool(name="depp", bufs=2))
    tpose = ctx.enter_context(tc.tile_pool(name="tpose", bufs=4))
    psum_t = ctx.enter_context(tc.tile_pool(name="psum_t", bufs=2, space="PSUM"))
    psum_e = ctx.enter_context(tc.tile_pool(name="psum_e", bufs=1, space="PSUM"))
    argp = ctx.enter_context(tc.tile_pool(name="argp", bufs=2))

    ident = consts.tile([P, P], F32, tag="ident")
    make_identity(nc, ident[:])

    # two load pieces per array; batch 0 first so its chunk pipeline starts
    # ~12us in, then batch 1's arrays.
    PIECES = [(0, 1024), (1024, HW - 1024)]
    TMAJ = 8  # tile-major over the last TMAJ chunks (tpose bufs = TMAJ+1)

    def alloc_tiles(b):
        rgb_t = [
            rgbp.tile([P, HW], F32, tag=f"rgb{t}", name=f"rgb_b{b}t{t}")
            for t in range(NT)
        ]
        dep_t = [
            depp.tile([P, HW], F32, tag=f"dep{t}", name=f"dep_b{b}t{t}")
            for t in range(NT)
        ]
        return rgb_t, dep_t

    def emit_loads(b, tiles, c0, w):
        rgb_t, dep_t = tiles
        for t in range(NT):
            row = b * C + t * P
            nc.sync.dma_start(
                rgb_t[t][:, c0 : c0 + w], rgb_d[row : row + P, c0 : c0 + w]
            )
        for t in range(NT):
            row = b * C + t * P
            nc.scalar.dma_start(
                dep_t[t][:, c0 : c0 + w], dep_d[row : row + P, c0 : c0 + w]
            )

    def emit_chunks(b, tiles):
        rgb_t, dep_t = tiles
        energy = [
            psum_e.tile([P, C], F32, tag=f"energy{t}", name=f"energy_b{b}t{t}")
            for t in range(NT)
        ]
        ops = [None] * NCH

        def emit_transposes(ch):
            cs = slice(ch * P, (ch + 1) * P)
            # fp32 transposes of the raw inputs (pure data movement):
            # ps_q = qT, ps_k = kT -- one PSUM bank each.
            ps_q = psum_t.tile([P, C], F32, tag="ps_q", name=f"ps_q_b{b}c{ch}")
            ps_k = psum_t.tile([P, C], F32, tag="ps_k", name=f"ps_k_b{b}c{ch}")
            for t in range(NT):
                nc.tensor.transpose(ps_q[:, t * P : (t + 1) * P], rgb_t[t][:, cs], ident[:])
                nc.tensor.transpose(ps_k[:, t * P : (t + 1) * P], dep_t[t][:, cs], ident[:])
            # fp16 main operands, derived in the casts:
            #   qh16s = fp16(2048 q) = 2048*qh   (exact power-of-2 scaling)
            #   kh16  = fp16(k)      = kh
            # main matmul qh16s.T @ kh16 accumulates at the 2^11 scale.
            qh16s = tpose.tile([P, C], F16, tag="qh16s", bufs=TMAJ + 1,
                               name=f"qh16s_b{b}c{ch}")
            kh16 = tpose.tile([P, C], F16, tag="kh16", bufs=TMAJ + 1,
                              name=f"kh16_b{b}c{ch}")
            kh16s = tpose.tile([P, C], F16, tag="kh16s", bufs=4,
                               name=f"kh16s_b{b}c{ch}")
            nc.vector.tensor_scalar_mul(qh16s[:], ps_q[:], SCALE)
            nc.scalar.copy(kh16[:], ps_k[:])
            # kh16s (s_t_t subtrahend only) alternates engines for balance:
            # DVE carries 4 ops on even chunks, ACT 4 on odd.
            if ch % 2 == 0:
                nc.vector.tensor_scalar_mul(kh16s[:], ps_k[:], SCALE)
            else:
                nc.scalar.mul(kh16s[:], ps_k[:], SCALE)
            # fp8 packs for the DoubleRow cross matmul:
            #   q8 = [e4m3(q)        | e4m3(2048(q-qh))]
            #   k8 = [e4m3(2048(k-kh)) | e4m3(k)]
            # lows via scalar_tensor_tensor (x*2048 - xh16s), highs as plain
            # casts. Using full q/k in the high slots only adds back an
            # O(ql*kl) term (~1e-6).
            q8 = tpose.tile([P, 2 * C], F8, tag="q8", bufs=TMAJ + 1,
                            name=f"q8_b{b}c{ch}")
            k8 = tpose.tile([P, 2 * C], F8, tag="k8", bufs=TMAJ + 1,
                            name=f"k8_b{b}c{ch}")
            nc.vector.scalar_tensor_tensor(
                q8[:, C : 2 * C], ps_q[:], SCALE, qh16s[:],
                op0=mybir.AluOpType.mult, op1=mybir.AluOpType.subtract,
            )
            nc.vector.scalar_tensor_tensor(
                k8[:, 0:C], ps_k[:], SCALE, kh16s[:],
                op0=mybir.AluOpType.mult, op1=mybir.AluOpType.subtract,
            )
            # highs from the fp16 copies (= e4m3(qh)/e4m3(kh), the exact
            # HW-verified path) rather than raw fp32
            nc.scalar.mul(q8[:, 0:C], qh16s[:], 1.0 / SCALE)
            nc.scalar.copy(k8[:, C : 2 * C], kh16[:])
            ops[ch] = (qh16s, kh16, q8, k8)

        def emit_matmuls(ch, tiles=range(NT)):
            qh16s, kh16, q8, k8 = ops[ch]
            q83 = q8[:].rearrange("p (two m) -> p two m", two=2)
            k83 = k8[:].rearrange("p (two n) -> p two n", two=2)
            for t in tiles:
                nc.tensor.matmul(
                    energy[t][:],
                    lhsT=qh16s[:, t * P : (t + 1) * P],
                    rhs=kh16[:],
                    start=(ch == 0),
                    stop=False,
                )
                nc.tensor.matmul(
                    energy[t][:],
                    lhsT=q83[:, :, t * P : (t + 1) * P],
                    rhs=k83,
                    start=False,
                    stop=(ch == NCH - 1),
                    perf_mode=mybir.MatmulPerfMode.DoubleRow,
                )

        emit_transposes(0)
        emit_transposes(1)
        emit_transposes(2)
        for ch in range(3, NCH):
            emit_transposes(ch)
            if ch - 3 < NCH - TMAJ:
                emit_matmuls(ch - 3)
        if NCH - 3 < NCH - TMAJ:
            emit_matmuls(NCH - 3)
        # tile-major for the last chunks: tile t's accumulation finishes early
        # so its argmax/gather/store overlaps the remaining matmuls.
        for t in range(NT):
            for ch in range(NCH - TMAJ, NCH):
                emit_matmuls(ch, tiles=[t])
        return energy

    def emit_tail(b, energy, rgb_t):
        for t in range(NT):
            _argmax_gather_store(
                nc, tc, argp, b, t, energy[t], rgb_t[t], dep_d, out_d
            )

    # Emission order matters: engine queues are IN ORDER, so b0's tail
    # (argmax on DVE -> gather descgen on Pool -> stores) is emitted right
    # after b0's chunks, BEFORE b1's chunk copies -- otherwise b0's
    # gathers/stores queue behind b1's whole energy phase (measured as a
    # 43us DMA idle gap). There are no split ops, so batch transitions on
    # DVE/ACT are burst-free.
    tiles0 = alloc_tiles(0)
    tiles1 = alloc_tiles(1)
    emit_loads(0, tiles0, *PIECES[0])
    emit_loads(0, tiles0, *PIECES[1])
    emit_loads(1, tiles1, *PIECES[0])
    emit_loads(1, tiles1, *PIECES[1])
    energy0 = emit_chunks(0, tiles0)
    emit_tail(0, energy0, tiles0[0])
    energy1 = emit_chunks(1, tiles1)
    emit_tail(1, energy1, tiles1[0])


@with_exitstack
def _body_fp16x3(ctx, tc, out_d, rgb_d, dep_d):
    nc = tc.nc
    consts = ctx.enter_context(tc.tile_pool(name="consts", bufs=1))
    rgbp = ctx.enter_context(tc.tile_pool(name="rgbp", bufs=2))
    depp = ctx.enter_context(tc.tile_pool(name="depp", bufs=2))
    splitp = ctx.enter_context(tc.tile_pool(name="splitp", bufs=1))
    tpose = ctx.enter_context(tc.tile_pool(name="tpose", bufs=5))
    psum_t = ctx.enter_context(tc.tile_pool(name="psum_t", bufs=2, space="PSUM"))
    psum_e = ctx.enter_context(tc.tile_pool(name="psum_e", bufs=1, space="PSUM"))
    argp = ctx.enter_context(tc.tile_pool(name="argp", bufs=2))

    ident = consts.tile([P, P], F16, tag="ident")
    make_identity(nc, ident[:])

    PIECES = [(0, 512), (512, HW - 512)]

    def emit_loads(b):
        rgb_t = []
        for t in range(NT):
            r = rgbp.tile([P, HW], F32, tag=f"rgb{t}", name=f"rgb_b{b}t{t}")
            rgb_t.append(r)
        for c0, w in PIECES:
            for t in range(NT):
                row = b * C + t * P
                nc.sync.dma_start(
                    rgb_t[t][:, c0 : c0 + w], rgb_d[row : row + P, c0 : c0 + w]
                )
        return rgb_t

    def emit_splits(b, rgb_t):
        qh_t, ql_t, kh_t, kl_t = [], [], [], []
        dls = []
        for t in range(NT):
            d = depp.tile([P, HW], F32, tag=f"dep{t % 2}", name=f"dep_b{b}t{t}")
            dls.append(d)
            qh_t.append(splitp.tile([P, HW], F16, tag=f"qh{t}", name=f"qh_b{b}t{t}"))
            ql_t.append(splitp.tile([P, HW], F16, tag=f"ql{t}", name=f"ql_b{b}t{t}"))
            kh_t.append(splitp.tile([P, HW], F16, tag=f"kh{t}", name=f"kh_b{b}t{t}"))
            kl_t.append(splitp.tile([P, HW], F16, tag=f"kl{t}", name=f"kl_b{b}t{t}"))
        for c0, w in PIECES:
            for t in range(NT):
                row = b * C + t * P
                nc.scalar.dma_start(
                    dls[t][:, c0 : c0 + w], dep_d[row : row + P, c0 : c0 + w]
                )
        for c0, w in PIECES:
            ps = slice(c0, c0 + w)
            for t in range(NT):
                nc.scalar.copy(qh_t[t][:, ps], rgb_t[t][:, ps])
                nc.vector.tensor_sub(ql_t[t][:, ps], rgb_t[t][:, ps], qh_t[t][:, ps])
                nc.scalar.copy(kh_t[t][:, ps], dls[t][:, ps])
                nc.gpsimd.tensor_sub(kl_t[t][:, ps], dls[t][:, ps], kh_t[t][:, ps])
        return qh_t, ql_t, kh_t, kl_t

    def emit_chunks(b, halves):
        qh_t, ql_t, kh_t, kl_t = halves
        energy = [
            psum_e.tile([P, C], F32, tag=f"energy{t}", name=f"energy_b{b}t{t}")
            for t in range(NT)
        ]
        qkT = [None] * NCH

        def emit_transposes(ch):
            cs = slice(ch * P, (ch + 1) * P)
            ps_q = psum_t.tile([P, 2 * C], F16, tag="ps_q", name=f"ps_q_b{b}c{ch}")
            ps_k = psum_t.tile([P, 2 * C], F16, tag="ps_k", name=f"ps_k_b{b}c{ch}")
            for t in range(NT):
                nc.tensor.transpose(ps_q[:, t * P : (t + 1) * P], qh_t[t][:, cs], ident[:])
                nc.tensor.transpose(ps_q[:, C + t * P : C + (t + 1) * P], ql_t[t][:, cs], ident[:])
                nc.tensor.transpose(ps_k[:, t * P : (t + 1) * P], kh_t[t][:, cs], ident[:])
                nc.tensor.transpose(ps_k[:, C + t * P : C + (t + 1) * P], kl_t[t][:, cs], ident[:])
            qT = tpose.tile([P, 2 * C], F16, tag="qT", bufs=7, name=f"qT_b{b}c{ch}")
            kT = tpose.tile([P, 2 * C], F16, tag="kT", bufs=6, name=f"kT_b{b}c{ch}")
            nc.vector.tensor_copy(qT[:], ps_q[:])
            nc.vector.tensor_copy(kT[:], ps_k[:])
            qkT[ch] = (qT, kT)

        def emit_matmuls(ch, tiles=range(NT)):
            qT, kT = qkT[ch]
            khT = kT[:, 0:C]
            klT = kT[:, C : 2 * C]
            for t in tiles:
                qhT_t = qT[:, t * P : (t + 1) * P]
                qlT_t = qT[:, C + t * P : C + (t + 1) * P]
                nc.tensor.matmul(energy[t][:], lhsT=qhT_t, rhs=khT,
                                 start=(ch == 0), stop=False)
                nc.tensor.matmul(energy[t][:], lhsT=qhT_t, rhs=klT,
                                 start=False, stop=False)
                nc.tensor.matmul(energy[t][:], lhsT=qlT_t, rhs=khT,
                                 start=False, stop=(ch == NCH - 1))

        TMAJ = 6
        emit_transposes(0)
        emit_transposes(1)
        emit_transposes(2)
        for ch in range(3, NCH):
            emit_transposes(ch)
            if ch - 3 < NCH - TMAJ:
                emit_matmuls(ch - 3)
        if NCH - 3 < NCH - TMAJ:
            emit_matmuls(NCH - 3)
        for t in range(NT):
            for ch in range(NCH - TMAJ, NCH):
                emit_matmuls(ch, tiles=[t])
        return energy

    def emit_tail(b, energy, rgb_t):
        for t in range(NT):
            _argmax_gather_store(
                nc, tc, argp, b, t, energy[t], rgb_t[t], dep_d, out_d
            )

    rgb0 = emit_loads(0)
    halves0 = emit_splits(0, rgb0)
    rgb1 = emit_loads(1)
    energy0 = emit_chunks(0, halves0)
    halves1 = emit_splits(1, rgb1)
    energy1 = emit_chunks(1, halves1)
    emit_tail(0, energy0, rgb0)
    emit_tail(1, energy1, rgb1)


def _build():
    nc = bacc.Bacc("TRN2", target_bir_lowering=False, debug=False)
    rgb_d = nc.dram_tensor("rgb", [NB * C, HW], F32, kind="ExternalInput")
    dep_d = nc.dram_tensor("depth", [NB * C, HW], F32, kind="ExternalInput")
    out_d = nc.dram_tensor("out", [NB * C, HW], F32, kind="ExternalOutput")
    body = _body_fp16_fp8dr if ENERGY_DT == "fp16_fp8dr" else _body_fp16x3
    with tile.TileContext(nc) as tc:
        body(tc, out_d.ap(), rgb_d.ap(), dep_d.ap())
    nc.compile()
    return nc


def get_nc():
    if "nc" not in _NC_CACHE:
        _NC_CACHE["nc"] = _build()
    return _NC_CACHE["nc"]


def make_in_maps(rgb, depth):
    rgb = np.ascontiguousarray(np.asarray(rgb, dtype=np.float32)).reshape(B, C, HW)
    depth = np.ascontiguousarray(np.asarray(depth, dtype=np.float32)).reshape(B, C, HW)
    in_maps = []
    for i in range(NCORES):
        sl = slice(i * NB, (i + 1) * NB)
        in_maps.append(
            {
                "rgb": np.ascontiguousarray(rgb[sl]).reshape(NB * C, HW),
                "depth": np.ascontiguousarray(depth[sl]).reshape(NB * C, HW),
            }
        )
    return in_maps


def kernel(rgb, depth):
    nc = get_nc()
    in_maps = make_in_maps(rgb, depth)
    res = run_bass_kernel_spmd(nc, in_maps, core_ids=list(range(NCORES)))
    outs = [res.results[i]["out"].reshape(NB, C, H, W) for i in range(NCORES)]
    return np.concatenate(outs, axis=0)


# revision 22
# speedup vs baseline: 1.3651x; 1.3414x over previous
"""Trainium2 Bass kernel for nn_CAM_Multimodal_Module (retrieval_knn).

Per batch b:
    energy[i, j] = <rgb[b, i, :], depth[b, j, :]>   (contraction over H*W)
    cl[i] = argmax_j energy[i, j]
    out[b, i, :] = rgb[b, i, :] + depth[b, cl[i], :]

Sharding: pure data parallel, 2 batches per core across 8 cores.

Energy path ("fp16_fp8dr", default): split q = qh + ql and k = kh + kl into
fp16 halves, then compute (at a global 2^11 scale, argmax-invariant)
    2^11 E ~= qh.(2^11 kh)  [fp16 matmul, 1 cyc/row]
            + [qh | 2^11 ql] .DR. [2^11 kl | kh]   [fp8 e4m3 DoubleRow, 0.5 cyc/row]
all accumulating into ONE PSUM bank per channel tile. The DoubleRow matmul
computes both cross terms (qh.kl + ql.kh, each carrying exactly one 2^11
factor) in a single 256-deep pass, halving the PE cost of the correction vs
the old fp16x3 scheme (verified offline on these inputs: 0 argmax flips,
top-2 margin 1.19e-3 vs fp64-truth gap 1.27e-3). Dropped ql.kl term is
O(1e-6). The scale assignment keeps every fp8 operand in e4m3's sweet range
(typ 0.3-1) with no separate-combine pass.

The exact add uses the original fp32 data: argmax indices drive gpsimd
indirect DMAs that gather exact fp32 depth rows from DRAM with a CCE add,
accumulating in-flight onto the fp32 rgb tiles (transfers are chunked to
4608 B -- larger indirect-DMA-with-add transfers corrupt on HW).

Set ENERGY_DT = "fp16x3" for the previous 3-matmul fp16 scheme.
"""

import numpy as np
from contextlib import ExitStack

import concourse.bass as bass
import concourse.tile as tile
from concourse import bacc, mybir
from concourse.bass_utils import run_bass_kernel_spmd
from concourse.masks import make_identity
from concourse._compat import with_exitstack

B, C, H, W = 16, 512, 48, 48
HW = H * W              # 2304
NCORES = 8
NB = B // NCORES        # 2 batches per core
P = 128
NT = C // P             # 4 channel tiles
NCH = HW // P           # 18 contraction chunks
F32 = mybir.dt.float32
F16 = mybir.dt.float16
F8 = mybir.dt.float8e4
SCALE = 2048.0          # 2^11

ENERGY_DT = "fp16_fp8dr"    # "fp16_fp8dr" | "fp16x3"

_NC_CACHE = {}


def _argmax_gather_store(nc, tc, argp, b, t, energy_t, rgb_t_t, dep_d, out_d):
    mx8 = argp.tile([P, 8], F32, tag="mx8", name=f"mx8_b{b}t{t}")
    nc.vector.max(mx8[:], energy_t[:])
    idx8 = argp.tile([P, 8], mybir.dt.uint32, tag="idx8", name=f"idx8_b{b}t{t}")
    nc.vector.max_index(idx8[:], mx8[:], energy_t[:])
    # gather exact fp32 depth rows from DRAM, accumulating onto the rgb tile
    # in-flight (CCE add). Transfers > 4608 B corrupt on HW, so chunk by 1152.
    half = HW // 2
    for c0 in (0, half):
        nc.gpsimd.indirect_dma_start(
            out=rgb_t_t[:, c0 : c0 + half],
            out_offset=None,
            in_=dep_d[:],
            in_offset=bass.IndirectOffsetOnAxis(ap=idx8[:, 0:1], axis=0),
            element_offset=b * C * HW + c0,
            compute_op=mybir.AluOpType.add,
        )
    # store on SP: a store DMA holds its dispatcher's SEQ while waiting on
    # the gather sems, which would head-of-line block compute dispatch
    nc.sync.dma_start(out_d[b * C + t * P : b * C + (t + 1) * P, :], rgb_t_t[:])


@with_exitstack
def _body_fp16_fp8dr(ctx, tc, out_d, rgb_d, dep_d):
    nc = tc.nc
    consts = ctx.enter_context(tc.tile_pool(name="consts", bufs=1))
    rgbp = ctx.enter_context(tc.tile_pool(name="rgbp", bufs=2))
    depp = ctx.enter_context(tc.tile_pool(name="depp", bufs=2))
    tpose = ctx.enter_context(tc.tile_pool(name="tpose", bufs=4))
    psum_t = ctx.enter_context(tc.tile_pool(name="psum_t", bufs=2, space="PSUM"))
    psum_e = ctx.enter_context(tc.tile_pool(name="psum_e", bufs=1, space="PSUM"))
    # bufs=4: with bufs=2 the argmax->gather chains of consecutive tiles
    # serialize on mx8/idx8 reuse (measured ~5us stalls between MaxIndex ops)
    argp = ctx.enter_context(tc.tile_pool(name="argp", bufs=4))

    ident = consts.tile([P, P], F32, tag="ident")
    make_identity(nc, ident[:])

    # two load pieces per array; batch 0 first so its chunk pipeline starts
    # ~12us in, then batch 1's arrays.
    # four load pieces: the chunk pipeline starts after piece0 (~6us) and
    # the load stream stays ahead of the transpose consumer thereafter
    PIECES = [(0, 512), (512, 512), (1024, 512), (1536, HW - 1536)]
    # tile-major depth for the last chunks (tpose bufs = TMAJ+1). Only the
    # LAST batch profits: its argmax/gather/store tail has nothing left to
    # hide behind, while an earlier batch's tail overlaps the next batch's
    # energy phase anyway -- and a deep tile-major region delays the next
    # batch's pipeline start on the in-order PE queue.
    TMAJ = 10
    TMAJ_BY_BATCH = {0: 2, 1: TMAJ}

    def alloc_tiles(b):
        rgb_t = [
            rgbp.tile([P, HW], F32, tag=f"rgb{t}", name=f"rgb_b{b}t{t}")
            for t in range(NT)
        ]
        dep_t = [
            depp.tile([P, HW], F32, tag=f"dep{t}", name=f"dep_b{b}t{t}")
            for t in range(NT)
        ]
        return rgb_t, dep_t

    def emit_loads(b, tiles, c0, w):
        # ALL loads on SP: DMA dispatches hold their SEQ under the in-flight
        # flow-control sems; on ACT that head-of-line blocks the chunk casts
        # (measured: a 12us all-engine stall while b1 loads drained).
        rgb_t, dep_t = tiles
        for t in range(NT):
            row = b * C + t * P
            nc.sync.dma_start(
                rgb_t[t][:, c0 : c0 + w], rgb_d[row : row + P, c0 : c0 + w]
            )
        for t in range(NT):
            row = b * C + t * P
            nc.sync.dma_start(
                dep_t[t][:, c0 : c0 + w], dep_d[row : row + P, c0 : c0 + w]
            )

    def emit_chunks(b, tiles):
        rgb_t, dep_t = tiles
        energy = [
            psum_e.tile([P, C], F32, tag=f"energy{t}", name=f"energy_b{b}t{t}")
            for t in range(NT)
        ]
        ops = [None] * NCH

        def emit_transposes(ch):
            cs = slice(ch * P, (ch + 1) * P)
            # fp32 transposes of the raw inputs (pure data movement):
            # ps_q = qT, ps_k = kT -- one PSUM bank each.
            ps_q = psum_t.tile([P, C], F32, tag="ps_q", name=f"ps_q_b{b}c{ch}")
            ps_k = psum_t.tile([P, C], F32, tag="ps_k", name=f"ps_k_b{b}c{ch}")
            for t in range(NT):
                nc.tensor.transpose(ps_q[:, t * P : (t + 1) * P], rgb_t[t][:, cs], ident[:])
                nc.tensor.transpose(ps_k[:, t * P : (t + 1) * P], dep_t[t][:, cs], ident[:])
            # fp16 main operands, derived in the casts:
            #   qh16s = fp16(2048 q) = 2048*qh   (exact power-of-2 scaling)
            #   kh16  = fp16(k)      = kh
            # main matmul qh16s.T @ kh16 accumulates at the 2^11 scale.
            qh16s = tpose.tile([P, C], F16, tag="qh16s", bufs=TMAJ + 1,
                               name=f"qh16s_b{b}c{ch}")
            kh16 = tpose.tile([P, C], F16, tag="kh16", bufs=TMAJ + 1,
                              name=f"kh16_b{b}c{ch}")
            kh16s = tpose.tile([P, C], F16, tag="kh16s", bufs=4,
                               name=f"kh16s_b{b}c{ch}")
            nc.vector.tensor_scalar_mul(qh16s[:], ps_q[:], SCALE)
            nc.scalar.copy(kh16[:], ps_k[:])
            nc.scalar.mul(kh16s[:], ps_k[:], SCALE)
            # fp8 packs for the DoubleRow cross matmul:
            #   q8 = [e4m3(q)        | e4m3(2048(q-qh))]
            #   k8 = [e4m3(2048(k-kh)) | e4m3(k)]
            # lows via scalar_tensor_tensor (x*2048 - xh16s), highs as plain
            # casts. Using full q/k in the high slots only adds back an
            # O(ql*kl) term (~1e-6).
            q8 = tpose.tile([P, 2 * C], F8, tag="q8", bufs=TMAJ + 1,
                            name=f"q8_b{b}c{ch}")
            k8 = tpose.tile([P, 2 * C], F8, tag="k8", bufs=TMAJ + 1,
                            name=f"k8_b{b}c{ch}")
            nc.vector.scalar_tensor_tensor(
                q8[:, C : 2 * C], ps_q[:], SCALE, qh16s[:],
                op0=mybir.AluOpType.mult, op1=mybir.AluOpType.subtract,
            )
            nc.vector.scalar_tensor_tensor(
                k8[:, 0:C], ps_k[:], SCALE, kh16s[:],
                op0=mybir.AluOpType.mult, op1=mybir.AluOpType.subtract,
            )
            # highs from the fp16 SBUF copies (= e4m3(qh)/e4m3(kh), the exact
            # HW-verified path) on gpsimd -- SBUF-to-SBUF, so the otherwise
            # idle Pool engine takes them, keeping ACT/DVE under the PE pace
            nc.gpsimd.tensor_scalar_mul(q8[:, 0:C], qh16s[:], 1.0 / SCALE)
            nc.gpsimd.tensor_copy(k8[:, C : 2 * C], kh16[:])
            ops[ch] = (qh16s, kh16, q8, k8)

        def emit_matmuls(ch, tiles=range(NT)):
            qh16s, kh16, q8, k8 = ops[ch]
            q83 = q8[:].rearrange("p (two m) -> p two m", two=2)
            k83 = k8[:].rearrange("p (two n) -> p two n", two=2)
            for t in tiles:
                nc.tensor.matmul(
                    energy[t][:],
                    lhsT=qh16s[:, t * P : (t + 1) * P],
                    rhs=kh16[:],
                    start=(ch == 0),
                    stop=False,
                )
                nc.tensor.matmul(
                    energy[t][:],
                    lhsT=q83[:, :, t * P : (t + 1) * P],
                    rhs=k83,
                    start=False,
                    stop=(ch == NCH - 1),
                    perf_mode=mybir.MatmulPerfMode.DoubleRow,
                )

        tmaj = TMAJ_BY_BATCH[b]
        emit_transposes(0)
        emit_transposes(1)
        emit_transposes(2)
        for ch in range(3, NCH):
            emit_transposes(ch)
            if ch - 3 < NCH - tmaj:
                emit_matmuls(ch - 3)
        if NCH - 3 < NCH - tmaj:
            emit_matmuls(NCH - 3)
        # tile-major for the last chunks: tile t's accumulation finishes early
        # so its argmax/gather/store overlaps the remaining matmuls.
        for t in range(NT):
            for ch in range(NCH - tmaj, NCH):
                emit_matmuls(ch, tiles=[t])
        return energy

    def emit_tail(b, energy, rgb_t):
        for t in range(NT):
            _argmax_gather_store(
                nc, tc, argp, b, t, energy[t], rgb_t[t], dep_d, out_d
            )

    # Emission order matters: engine queues are IN ORDER, so b0's tail
    # (argmax on DVE -> gather descgen on Pool -> stores) is emitted right
    # after b0's chunks, BEFORE b1's chunk copies -- otherwise b0's
    # gathers/stores queue behind b1's whole energy phase (measured as a
    # 43us DMA idle gap). There are no split ops, so batch transitions on
    # DVE/ACT are burst-free.
    tiles0 = alloc_tiles(0)
    tiles1 = alloc_tiles(1)
    for pc in PIECES:
        emit_loads(0, tiles0, *pc)
    for pc in PIECES:
        emit_loads(1, tiles1, *pc)
    energy0 = emit_chunks(0, tiles0)
    emit_tail(0, energy0, tiles0[0])
    energy1 = emit_chunks(1, tiles1)
    emit_tail(1, energy1, tiles1[0])


@with_exitstack
def _body_fp16x3(ctx, tc, out_d, rgb_d, dep_d):
    nc = tc.nc
    consts = ctx.enter_context(tc.tile_pool(name="consts", bufs=1))
    rgbp = ctx.enter_context(tc.tile_pool(name="rgbp", bufs=2))
    depp = ctx.enter_context(tc.tile_pool(name="depp", bufs=2))
    splitp = ctx.enter_context(tc.tile_pool(name="splitp", bufs=1))
    tpose = ctx.enter_context(tc.tile_pool(name="tpose", bufs=5))
    psum_t = ctx.enter_context(tc.tile_pool(name="psum_t", bufs=2, space="PSUM"))
    psum_e = ctx.enter_context(tc.tile_pool(name="psum_e", bufs=1, space="PSUM"))
    argp = ctx.enter_context(tc.tile_pool(name="argp", bufs=2))

    ident = consts.tile([P, P], F16, tag="ident")
    make_identity(nc, ident[:])

    PIECES = [(0, 512), (512, HW - 512)]

    def emit_loads(b):
        rgb_t = []
        for t in range(NT):
            r = rgbp.tile([P, HW], F32, tag=f"rgb{t}", name=f"rgb_b{b}t{t}")
            rgb_t.append(r)
        for c0, w in PIECES:
            for t in range(NT):
                row = b * C + t * P
                nc.sync.dma_start(
                    rgb_t[t][:, c0 : c0 + w], rgb_d[row : row + P, c0 : c0 + w]
                )
        return rgb_t

    def emit_splits(b, rgb_t):
        qh_t, ql_t, kh_t, kl_t = [], [], [], []
        dls = []
        for t in range(NT):
            d = depp.tile([P, HW], F32, tag=f"dep{t % 2}", name=f"dep_b{b}t{t}")
            dls.append(d)
            qh_t.append(splitp.tile([P, HW], F16, tag=f"qh{t}", name=f"qh_b{b}t{t}"))
            ql_t.append(splitp.tile([P, HW], F16, tag=f"ql{t}", name=f"ql_b{b}t{t}"))
            kh_t.append(splitp.tile([P, HW], F16, tag=f"kh{t}", name=f"kh_b{b}t{t}"))
            kl_t.append(splitp.tile([P, HW], F16, tag=f"kl{t}", name=f"kl_b{b}t{t}"))
        for c0, w in PIECES:
            for t in range(NT):
                row = b * C + t * P
                nc.scalar.dma_start(
                    dls[t][:, c0 : c0 + w], dep_d[row : row + P, c0 : c0 + w]
                )
        for c0, w in PIECES:
            ps = slice(c0, c0 + w)
            for t in range(NT):
                nc.scalar.copy(qh_t[t][:, ps], rgb_t[t][:, ps])
                nc.vector.tensor_sub(ql_t[t][:, ps], rgb_t[t][:, ps], qh_t[t][:, ps])
                nc.scalar.copy(kh_t[t][:, ps], dls[t][:, ps])
                nc.gpsimd.tensor_sub(kl_t[t][:, ps], dls[t][:, ps], kh_t[t][:, ps])
        return qh_t, ql_t, kh_t, kl_t

    def emit_chunks(b, halves):
        qh_t, ql_t, kh_t, kl_t = halves
        energy = [
            psum_e.tile([P, C], F32, tag=f"energy{t}", name=f"energy_b{b}t{t}")
            for t in range(NT)
        ]
        qkT = [None] * NCH

        def emit_transposes(ch):
            cs = slice(ch * P, (ch + 1) * P)
            ps_q = psum_t.tile([P, 2 * C], F16, tag="ps_q", name=f"ps_q_b{b}c{ch}")
            ps_k = psum_t.tile([P, 2 * C], F16, tag="ps_k", name=f"ps_k_b{b}c{ch}")
            for t in range(NT):
                nc.tensor.transpose(ps_q[:, t * P : (t + 1) * P], qh_t[t][:, cs], ident[:])
                nc.tensor.transpose(ps_q[:, C + t * P : C + (t + 1) * P], ql_t[t][:, cs], ident[:])
                nc.tensor.transpose(ps_k[:, t * P : (t + 1) * P], kh_t[t][:, cs], ident[:])
                nc.tensor.transpose(ps_k[:, C + t * P : C + (t + 1) * P], kl_t[t][:, cs], ident[:])
            qT = tpose.tile([P, 2 * C], F16, tag="qT", bufs=7, name=f"qT_b{b}c{ch}")
            kT = tpose.tile([P, 2 * C], F16, tag="kT", bufs=6, name=f"kT_b{b}c{ch}")
            nc.vector.tensor_copy(qT[:], ps_q[:])
            nc.vector.tensor_copy(kT[:], ps_k[:])
            qkT[ch] = (qT, kT)

        def emit_matmuls(ch, tiles=range(NT)):
            qT, kT = qkT[ch]
            khT = kT[:, 0:C]
            klT = kT[:, C : 2 * C]
            for t in tiles:
                qhT_t = qT[:, t * P : (t + 1) * P]
                qlT_t = qT[:, C + t * P : C + (t + 1) * P]
                nc.tensor.matmul(energy[t][:], lhsT=qhT_t, rhs=khT,
                                 start=(ch == 0), stop=False)
                nc.tensor.matmul(energy[t][:], lhsT=qhT_t, rhs=klT,
                                 start=False, stop=False)
                nc.tensor.matmul(energy[t][:], lhsT=qlT_t, rhs=khT,
                                 start=False, stop=(ch == NCH - 1))

        TMAJ = 6
        emit_transposes(0)
        emit_transposes(1)
        emit_transposes(2)
        for ch in range(3, NCH):
            emit_transposes(ch)
            if ch - 3 < NCH - TMAJ:
                emit_matmuls(ch - 3)
        if NCH - 3 < NCH - TMAJ:
            emit_matmuls(NCH - 3)
        for t in range(NT):
            for ch in range(NCH - TMAJ, NCH):
                emit_matmuls(ch, tiles=[t])
        return energy

    def emit_tail(b, energy, rgb_t):
        for t in range(NT):
            _argmax_gather_store(
                nc, tc, argp, b, t, energy[t], rgb_t[t], dep_d, out_d
            )

    rgb0 = emit_loads(0)
    halves0 = emit_splits(0, rgb0)
    rgb1 = emit_loads(1)
    energy0 = emit_chunks(0, halves0)
    halves1 = emit_splits(1, rgb1)
    energy1 = emit_chunks(1, halves1)
    emit_tail(0, energy0, rgb0)
    emit_tail(1, energy1, rgb1)


def _build():
    nc = bacc.Bacc("TRN2", target_bir_lowering=False, debug=False)
    rgb_d = nc.dram_tensor("rgb", [NB * C, HW], F32, kind="ExternalInput")
    dep_d = nc.dram_tensor("depth", [NB * C, HW], F32, kind="ExternalInput")
    out_d = nc.dram_tensor("out", [NB * C, HW], F32, kind="ExternalOutput")
    body = _body_fp16_fp8dr if ENERGY_DT == "fp16_fp8dr" else _body_fp16x3
    with tile.TileContext(nc) as tc:
        body(tc, out_d.ap(), rgb_d.ap(), dep_d.ap())
    nc.compile()
    return nc


def get_nc():
    if "nc" not in _NC_CACHE:
        _NC_CACHE["nc"] = _build()
    return _NC_CACHE["nc"]


def make_in_maps(rgb, depth):
    rgb = np.ascontiguousarray(np.asarray(rgb, dtype=np.float32)).reshape(B, C, HW)
    depth = np.ascontiguousarray(np.asarray(depth, dtype=np.float32)).reshape(B, C, HW)
    in_maps = []
    for i in range(NCORES):
        sl = slice(i * NB, (i + 1) * NB)
        in_maps.append(
            {
                "rgb": np.ascontiguousarray(rgb[sl]).reshape(NB * C, HW),
                "depth": np.ascontiguousarray(depth[sl]).reshape(NB * C, HW),
            }
        )
    return in_maps


def kernel(rgb, depth):
    nc = get_nc()
    in_maps = make_in_maps(rgb, depth)
    res = run_bass_kernel_spmd(nc, in_maps, core_ids=list(range(NCORES)))
    outs = [res.results[i]["out"].reshape(NB, C, H, W) for i in range(NCORES)]
    return np.concatenate(outs, axis=0)
